# revision 2
# baseline (speedup 1.0000x reference)
"""AFNO1D Trainium2 kernel v3.

v2 + fixes for the GPSIMD-cannot-access-PSUM constraint, plus:

- L3 residual folded into the matmul streams: an identity matmul (bf16,
  weight 65536*I) accumulates 65536*u into pA and 65536*v into pB where
  u=(xl+xh)/2, v=(xl-xh)/2 are host-precomputed. Then
  outl_raw = pA+pB, outh_raw = pA-pB (one tensor_tensor each, DVE) and
  the host applies the exact 2^-16 descale (pure exponent shift).
- L2 softshrink forms, chosen per tensor to balance engines:
    A: p = Act-relu(q + SO*(b2-lam));  o2 = DVE stt min(q + SO*(b2+lam), p)
    E: zc = DVE ts copy(q) -> bf16;    Pool: m = ts min(zc+bp, 0),
                                       o2 = stt max(zc + bm, m)
    F: zc = Act-copy(q) -> bf16;       Pool ops as E.
  (softshrink(z) = max(z-lam, min(z+lam, 0)) = min(z+lam, relu(z-lam)))
- Pool engine only ever touches SBUF.
"""

from contextlib import ExitStack

import numpy as np
import ml_dtypes

import concourse.bass as bass
import concourse.mybir as mybir
import concourse.tile as tile
from concourse import bacc
from concourse.bass_utils import run_bass_kernel_spmd

HIDDEN = 1024
NB = 8          # channel blocks
NBH = NB // 2
BS = 128        # block size
C2 = HIDDEN // 2
LAM = 0.01
N_CORES = 8
NROWS = 4096    # rows (sequence positions) per core
R = 512         # rows per chunk
NCHUNK = NROWS // R

SX = 8.0        # x -> fp8 scale
SW1 = 512.0     # W1_eff -> fp8 scale
SO = 32.0       # o1 and o2 -> fp8 scale
SG = 2048.0     # IFFT cos/sin -> fp8 scale; SO*SG = 2^16
SR = SO * SG    # residual pre-scale for the identity matmul
ALU = mybir.AluOpType

F32 = mybir.dt.float32
BF16 = mybir.dt.bfloat16
E4 = mybir.dt.float8e4
DR = mybir.MatmulPerfMode.DoubleRow
RELU = mybir.ActivationFunctionType.Relu
COPY = mybir.ActivationFunctionType.Copy

# per-(kb,ri) softshrink form, index = kb*2 + ri.
# A: p = Act-relu(q + bm); o2 = DVE stt min(q + bp, p)   (2 ops, no Pool)
# T: zm = DVE ts (q + bm) -> bf16; m = Pool ts min(zm + 2lam, 0);
#    o2 = DVE stt max(zm + 0, m)  [all-imm/1-AP ops: Pool-legal]
FORMS = "AAAAAAAAAAAAAAAA"

_GRAPH_CACHE = {}


def _build_graph(rep=1):
    key = ("v6", rep)
    if key in _GRAPH_CACHE:
        return _GRAPH_CACHE[key]

    nc = bacc.Bacc("TRN2", target_bir_lowering=False, debug=False,
                   num_devices=N_CORES)

    xt8 = nc.dram_tensor("xt8", [BS, NB, NROWS], E4, kind="ExternalInput").ap()
    uv = nc.dram_tensor("uv", [BS, 2, NBH, NROWS], BF16, kind="ExternalInput").ap()
    x512 = nc.dram_tensor("x512", [1, NROWS], F32, kind="ExternalInput").ap()
    w1r = nc.dram_tensor("w1r", [BS, NB, HIDDEN], E4, kind="ExternalInput").ap()
    w1i = nc.dram_tensor("w1i", [BS, NB, HIDDEN], E4, kind="ExternalInput").ap()
    w2rp = nc.dram_tensor("w2rp", [BS, NB, 2, BS], E4, kind="ExternalInput").ap()
    w2ip = nc.dram_tensor("w2ip", [BS, NB, 2, BS], E4, kind="ExternalInput").ap()
    gcp = nc.dram_tensor("gcp", [BS, NBH, 2, C2], E4, kind="ExternalInput").ap()
    gsp = nc.dram_tensor("gsp", [BS, NBH, 2, C2], E4, kind="ExternalInput").ap()
    g512 = nc.dram_tensor("g512", [BS, NB], E4, kind="ExternalInput").ap()
    idw = nc.dram_tensor("idw", [BS, BS], BF16, kind="ExternalInput").ap()
    # bias columns: b1r, b1i, s2mr, s2mi, s2pr, s2pi (each [BS, NB])
    biases = nc.dram_tensor("biases", [BS, 6, NB], F32, kind="ExternalInput").ap()

    # outl/outh hold 65536*(out chunk) in bf16; host shifts the exponent back
    outl = nc.dram_tensor("outl", [BS, NBH, NROWS], BF16, kind="ExternalOutput").ap()
    outh = nc.dram_tensor("outh", [BS, NBH, NROWS], BF16, kind="ExternalOutput").ap()
    out5 = nc.dram_tensor("out5", [1, NROWS], F32, kind="ExternalOutput").ap()

    SIG = 1.0 / (SX * SW1)   # L1 psum descale (o1 at true scale)
    K5 = 1.0 / (SO * 32.0)   # channel-512 descale
    LAMS = LAM * SO          # lambda in the SO-scaled domain

    with tile.TileContext(nc) as tc, ExitStack() as ctx:
        wpool = ctx.enter_context(tc.tile_pool(name="weights", bufs=1))
        w1r_sb = wpool.tile([BS, NB, HIDDEN], E4, tag="w1r", name="w1r_sb")
        w1i_sb = wpool.tile([BS, NB, HIDDEN], E4, tag="w1i", name="w1i_sb")
        bias_sb = wpool.tile([BS, 6, NB], F32, tag="biases", name="bias_sb")
        nc.sync.dma_start(out=w1r_sb[:], in_=w1r[:])
        nc.sync.dma_start(out=w1i_sb[:], in_=w1i[:])
        nc.sync.dma_start(out=bias_sb[:], in_=biases[:])
        b1r = bias_sb[:, 0, :]
        b1i = bias_sb[:, 1, :]
        s2m = (bias_sb[:, 2, :], bias_sb[:, 3, :])   # SO*(b2 - lam), r/i
        s2p = (bias_sb[:, 4, :], bias_sb[:, 5, :])   # SO*(b2 + lam), r/i

        w2rp_sb = wpool.tile([BS, NB, 2, BS], E4, tag="w2rp", name="w2rp_sb")
        w2ip_sb = wpool.tile([BS, NB, 2, BS], E4, tag="w2ip", name="w2ip_sb")
        gcp_sb = wpool.tile([BS, NBH, 2, C2], E4, tag="gcp", name="gcp_sb")
        gsp_sb = wpool.tile([BS, NBH, 2, C2], E4, tag="gsp", name="gsp_sb")
        g512_sb = wpool.tile([BS, NB], E4, tag="g512", name="g512_sb")
        idw_sb = wpool.tile([BS, BS], BF16, tag="idw", name="idw_sb")

        def _late_weight_dmas():
            nc.sync.dma_start(out=w2rp_sb[:], in_=w2rp[:])
            nc.sync.dma_start(out=w2ip_sb[:], in_=w2ip[:])
            nc.sync.dma_start(out=gcp_sb[:], in_=gcp[:])
            nc.sync.dma_start(out=gsp_sb[:], in_=gsp[:])
            nc.sync.dma_start(out=g512_sb[:], in_=g512[:])
            nc.sync.dma_start(out=idw_sb[:], in_=idw[:])

        xqpool = ctx.enter_context(tc.tile_pool(name="xq", bufs=2))
        xpool = ctx.enter_context(tc.tile_pool(name="xin", bufs=2))
        opool = ctx.enter_context(tc.tile_pool(name="acts", bufs=2))
        bfpool = ctx.enter_context(tc.tile_pool(name="bf", bufs=3))
        outpool = ctx.enter_context(tc.tile_pool(name="outs", bufs=2))
        ppool = ctx.enter_context(tc.tile_pool(name="psum", bufs=4, space="PSUM"))

        NITER = NCHUNK * rep
        saved = {}   # ch -> (r0, o2r, o2i, uv_sb, x5_sb)

        def emit_front(ch):
            r0 = (ch % NCHUNK) * R
            xt_q = xqpool.tile([BS, NB, R], E4, tag="xt_q", name=f"xt_q{ch}")
            nc.sync.dma_start(out=xt_q[:], in_=xt8[:, :, r0:r0 + R])
            uv_sb = xpool.tile([BS, 2, NBH, R], BF16, tag="uv", name=f"uv{ch}")
            nc.sync.dma_start(out=uv_sb[:], in_=uv[:, :, :, r0:r0 + R])
            x5_sb = xpool.tile([1, R], F32, tag="x512", name=f"x512_{ch}")
            nc.sync.dma_start(out=x5_sb[:], in_=x512[0:1, r0:r0 + R])
            if ch == 0:
                _late_weight_dmas()

            # ---- layer 1 (FFT fused): o1 = relu(SIG*psum + b1), paired fp8
            o1p = opool.tile([BS, NB, 2, R], E4, tag="o1p", name=f"o1p{ch}")
            for ro in range(NB):
                pr = ppool.tile([BS, R], F32, tag="pr", name=f"pr{ch}_{ro}")
                pi = ppool.tile([BS, R], F32, tag="pi", name=f"pi{ch}_{ro}")
                cs = slice(ro * BS, (ro + 1) * BS)
                for t in range(NBH):
                    nc.tensor.matmul(pr[:], w1r_sb[:, 2 * t:2 * t + 2, cs],
                                     xt_q[:, 2 * t:2 * t + 2, :],
                                     start=(t == 0), stop=(t == NBH - 1),
                                     perf_mode=DR)
                for t in range(NBH):
                    nc.tensor.matmul(pi[:], w1i_sb[:, 2 * t:2 * t + 2, cs],
                                     xt_q[:, 2 * t:2 * t + 2, :],
                                     start=(t == 0), stop=(t == NBH - 1),
                                     perf_mode=DR)
                nc.scalar.activation(o1p[:, ro, 0, :], pr[:], RELU, scale=SIG,
                                     bias=b1r[:, ro:ro + 1])
                nc.scalar.activation(o1p[:, ro, 1, :], pi[:], RELU, scale=SIG,
                                     bias=b1i[:, ro:ro + 1])

            # ---- layer 2 (block-diag complex) + softshrink
            o2r = opool.tile([BS, NB, R], E4, tag="o2r", name=f"o2r{ch}")
            o2i = opool.tile([BS, NB, R], E4, tag="o2i", name=f"o2i{ch}")
            for kb in range(NB):
                qr = ppool.tile([BS, R], F32, tag="pr", name=f"qr{ch}_{kb}")
                qi = ppool.tile([BS, R], F32, tag="pi", name=f"qi{ch}_{kb}")
                nc.tensor.matmul(qr[:], w2rp_sb[:, kb, :, :], o1p[:, kb, :, :],
                                 start=True, stop=True, perf_mode=DR)
                nc.tensor.matmul(qi[:], w2ip_sb[:, kb, :, :], o1p[:, kb, :, :],
                                 start=True, stop=True, perf_mode=DR)
                for ri, (q, o2) in enumerate(((qr, o2r), (qi, o2i))):
                    bm = s2m[ri][:, kb:kb + 1]
                    bp = s2p[ri][:, kb:kb + 1]
                    form = FORMS[kb * 2 + ri]
                    if form == "A":
                        p = bfpool.tile([BS, R], BF16, tag=f"p{ri}",
                                        name=f"p{ch}_{kb}_{ri}")
                        nc.scalar.activation(p[:], q[:], RELU, bias=bm)
                        nc.vector.scalar_tensor_tensor(o2[:, kb, :], q[:], bp,
                                                       p[:], ALU.add, ALU.min)
                    else:   # T: zm = q + SO*(b2-lam); m = min(zm+2lam', 0);
                            #    o2 = max(zm, m)
                        zm = bfpool.tile([BS, R], BF16, tag=f"z{ri}",
                                         name=f"z{ch}_{kb}_{ri}")
                        nc.vector.tensor_scalar(zm[:], q[:], bm, None, ALU.add)
                        m = bfpool.tile([BS, R], BF16, tag=f"m{ri}",
                                        name=f"m{ch}_{kb}_{ri}")
                        nc.gpsimd.tensor_scalar(m[:], zm[:], 2.0 * LAMS, 0.0,
                                                ALU.add, ALU.min)
                        nc.vector.tensor_tensor(o2[:, kb, :], zm[:], m[:],
                                                ALU.max)
            saved[ch] = (r0, o2r, o2i, uv_sb, x5_sb)

        def emit_back(ch):
            # ---- layer 3 (IFFT real part, half-spectrum), residual via
            # identity matmul: pA = 2^16*(A + u), pB = 2^16*(B + v);
            # outl = pA + pB, outh = pA - pB (pA evacuated to SBUF first:
            # only one PSUM operand allowed per instruction)
            r0, o2r, o2i, uv_sb, x5_sb = saved.pop(ch)
            outl_f = outpool.tile([BS, NBH, R], BF16, tag="outl_f",
                                  name=f"outl_f{ch}")
            outh_f = outpool.tile([BS, NBH, R], BF16, tag="outh_f",
                                  name=f"outh_f{ch}")
            for co in range(NBH):
                pA = ppool.tile([BS, R], F32, tag="pr", name=f"pA{ch}_{co}")
                pB = ppool.tile([BS, R], F32, tag="pi", name=f"pB{ch}_{co}")
                cs = slice(co * BS, (co + 1) * BS)
                for t in range(NBH):
                    nc.tensor.matmul(pA[:], gcp_sb[:, t, :, cs],
                                     o2r[:, 2 * t:2 * t + 2, :],
                                     start=(t == 0), stop=False,
                                     perf_mode=DR)
                nc.tensor.matmul(pA[:], idw_sb[:], uv_sb[:, 0, co, :],
                                 start=False, stop=True)
                for t in range(NBH):
                    nc.tensor.matmul(pB[:], gsp_sb[:, t, :, cs],
                                     o2i[:, 2 * t:2 * t + 2, :],
                                     start=(t == 0), stop=False,
                                     perf_mode=DR)
                nc.tensor.matmul(pB[:], idw_sb[:], uv_sb[:, 1, co, :],
                                 start=False, stop=True)
                a_sb = bfpool.tile([BS, R], BF16, tag="a_sb",
                                   name=f"a_sb{ch}_{co}")
                if co % 2 == 0:
                    nc.scalar.activation(a_sb[:], pA[:], COPY)
                else:
                    nc.vector.tensor_scalar(a_sb[:], pA[:], 0.0, None, ALU.add)
                nc.vector.tensor_tensor(outl_f[:, co, :], a_sb[:], pB[:],
                                        ALU.add)
                nc.vector.scalar_tensor_tensor(outh_f[:, co, :], pB[:], -1.0,
                                               a_sb[:], ALU.mult, ALU.add)
            nc.sync.dma_start(out=outl[:, :, r0:r0 + R], in_=outl_f[:])
            nc.sync.dma_start(out=outh[:, :, r0:r0 + R], in_=outh_f[:])

            # channel 512: out[512] = K5 * sum_k 32*(+-1)(SO*o2r[k]) + x[512]
            p5 = ppool.tile([1, R], F32, tag="pi", name=f"p5_{ch}")
            for kb in range(NB):
                nc.tensor.matmul(p5[:], g512_sb[:, kb:kb + 1],
                                 o2r[:, kb, :],
                                 start=(kb == 0), stop=(kb == NB - 1))
            o5 = outpool.tile([1, R], F32, tag="out5_f", name=f"o5_{ch}")
            nc.vector.scalar_tensor_tensor(o5[:], p5[:], K5, x5_sb[:],
                                           ALU.mult, ALU.add)
            nc.sync.dma_start(out=out5[0:1, r0:r0 + R], in_=o5[:])

        # software pipeline with one-chunk skew: L3(ch-1) is emitted after
        # L1/L2(ch) so the in-order PE queue always has ready matmuls.
        for it in range(NITER + 1):
            if it < NITER:
                emit_front(it)
            if it >= 1:
                emit_back(it - 1)

    nc.compile()
    _GRAPH_CACHE[key] = nc
    return nc


def _build_host_weights(w1, b1, w2, b2):
    C = HIDDEN
    k = np.arange(C)
    c = np.arange(C)
    ph = (np.outer(c, k) % C).astype(np.float64) * (2.0 * np.pi / C)
    s = 1.0 / np.sqrt(C)
    Fr = np.cos(ph) * s        # [c, k]
    Fi = -np.sin(ph) * s
    w1 = np.asarray(w1, np.float64)
    W1r = np.empty((C, C), np.float64)
    W1i = np.empty((C, C), np.float64)
    for kb in range(NB):
        cols = slice(kb * BS, (kb + 1) * BS)
        W1r[:, cols] = Fr[:, cols] @ w1[0, kb] - Fi[:, cols] @ w1[1, kb]
        W1i[:, cols] = Fi[:, cols] @ w1[0, kb] + Fr[:, cols] @ w1[1, kb]
    Gr = Fr.T.copy()           # cos(2pi k c / C)/sqrt(C)
    Gi = Fi.T.copy()           # -sin(2pi k c / C)/sqrt(C)

    b1 = np.asarray(b1, np.float64)
    b2 = np.asarray(b2, np.float64)
    w2 = np.asarray(w2, np.float64)
    f8 = ml_dtypes.float8_e4m3
    bf = ml_dtypes.bfloat16

    W1rs = (W1r * SW1).reshape(NB, BS, HIDDEN).transpose(1, 0, 2)
    W1is = (W1i * SW1).reshape(NB, BS, HIDDEN).transpose(1, 0, 2)

    Ghc = (Gr[:, :C2] * SG).reshape(NB, BS, C2)
    Ghs = (Gi[:, :C2] * SG).reshape(NB, BS, C2)      # Gi = -sin -> pB = -SR*B
    gcp = np.stack([Ghc[0::2], Ghc[1::2]], axis=2).transpose(1, 0, 2, 3)
    gsp = np.stack([Ghs[0::2], Ghs[1::2]], axis=2).transpose(1, 0, 2, 3)

    w2r0, w2r1 = w2[0] * SO, -w2[1] * SO             # [NB, BS, BS] each
    w2i0, w2i1 = w2[1] * SO, w2[0] * SO
    w2rp = np.stack([w2r0, w2r1], axis=2).transpose(1, 0, 2, 3)
    w2ip = np.stack([w2i0, w2i1], axis=2).transpose(1, 0, 2, 3)

    # out[512] = (1/sqrt(C)) * sum_k (-1)^k o2_true[k]; the 1/32 lives in K5
    alt = ((-1.0) ** k).reshape(NB, BS)              # [kb, p]
    g512p = np.ascontiguousarray(alt.T)              # [BS, NB]

    biases = np.stack([
        b1[0].T, b1[1].T,
        (SO * (b2[0] - LAM)).T, (SO * (b2[1] - LAM)).T,
        (SO * (b2[0] + LAM)).T, (SO * (b2[1] + LAM)).T,
    ], axis=1)                                       # [BS, 6, NB]

    return {
        "w1r": np.ascontiguousarray(W1rs).astype(f8),
        "w1i": np.ascontiguousarray(W1is).astype(f8),
        "gcp": np.ascontiguousarray(gcp).astype(f8),
        "gsp": np.ascontiguousarray(gsp).astype(f8),
        "g512": np.ascontiguousarray(g512p).astype(f8),
        "w2rp": np.ascontiguousarray(w2rp).astype(f8),
        "w2ip": np.ascontiguousarray(w2ip).astype(f8),
        "idw": np.ascontiguousarray(SR * np.eye(BS)).astype(bf),
        "biases": np.ascontiguousarray(biases).astype(np.float32),
    }


def _make_in_maps(x, w1, b1, w2, b2):
    x = np.asarray(x, np.float32)
    B = x.shape[0]
    weights = _build_host_weights(w1, b1, w2, b2)
    in_maps = []
    f8 = ml_dtypes.float8_e4m3
    bf = ml_dtypes.bfloat16
    rev_idx = (HIDDEN - np.arange(C2)) % HIDDEN      # c -> 1024-c
    for b in range(B):
        m = dict(weights)
        xt_b = np.ascontiguousarray(x[b].T)          # [C, NROWS]
        xpm = xt_b.reshape(NB, BS, NROWS).transpose(1, 0, 2)
        m["xt8"] = np.ascontiguousarray(xpm * SX).astype(f8)
        xl = xpm[:, :NBH, :].astype(np.float32)      # [BS, NBH, NROWS]
        xh = xt_b[rev_idx].reshape(NBH, BS, NROWS).transpose(1, 0, 2)
        u = (xl + xh) * 0.5
        v = (xl - xh) * 0.5
        m["uv"] = np.ascontiguousarray(
            np.stack([u, v], axis=1)).astype(bf)     # [BS, 2, NBH, NROWS]
        m["x512"] = np.ascontiguousarray(xt_b[C2:C2 + 1]).astype(np.float32)
        in_maps.append(m)
    return in_maps


def _run(x, w1, b1, w2, b2, trace=False):
    nc = _build_graph()
    x = np.asarray(x, np.float32)
    B = x.shape[0]
    in_maps = _make_in_maps(x, w1, b1, w2, b2)
    res = run_bass_kernel_spmd(nc, in_maps, core_ids=list(range(N_CORES)),
                               trace=trace)
    outs = np.empty_like(x)
    out_t = np.empty((HIDDEN, NROWS), np.float32)
    K3 = np.float32(1.0 / SR)
    for b in range(B):
        r = res.results[b]
        lo = r["outl"].astype(np.float32).transpose(1, 0, 2).reshape(C2, NROWS)
        hi = r["outh"].astype(np.float32).transpose(1, 0, 2).reshape(C2, NROWS)
        out_t[:C2] = lo * K3
        out_t[C2] = r["out5"][0]
        out_t[C2 + 1:] = hi[1:][::-1] * K3
        outs[b] = out_t.T
    return outs, res


def kernel(x, w1, b1, w2, b2):
    outs, _ = _run(x, w1, b1, w2, b2, trace=False)
    return outs
